# revision 21
# baseline (speedup 1.0000x reference)
"""Depthwise 1d (per-channel linear) Trainium2 Bass kernel.

out[n, c, o] = sum_i x[n, c, i] * W[c, o, i] + b[c, o]
  x: [4096, 256, 64] f32, W: [256, 128, 64] f32, b: [256, 128] f32
  out: [4096, 256, 128] f32

Strategy: shard channels C across 8 cores (32 ch/core, full batch).
Channels are fully independent, so there are no collectives; sharding C
instead of N means each core only needs 1/8th of the weights.

The kernel is DMA-bound (per-core HBM ~332 GB/s), so the whole design
minimizes HBM bytes:
  * x is downcast to fp16 AND pre-transposed on the host into the exact
    SBUF layout the PE wants: [k, tile, pair, n] where partitions k<64
    hold channel 2p's 64 taps and k>=64 hold channel 2p+1's. No on-device
    transposes, no hi/lo split -- halves x traffic vs f32.
  * the output is stored as fp16 and upcast to f32 on the host -- halves
    out traffic. (Error budget: gate is 2e-2; fp16 end-to-end is ~1.4e-3.)
  * weights are uploaded as the fully-assembled block-diagonal fp16 tiles
    [k, pair, 2*HO] (upper-left = W_even.T, lower-right = W_odd.T) so one
    matmul per channel pair contracts the full 128 partitions.

Per n-tile (128 rows) x pair: out_pair = xt_pair.T @ wt_pair, a
[128,128]x[128,256] fp16 matmul accumulated in fp32 PSUM. Each pair's
accumulation group is opened by a bias matmul: ones[128,128].T @
wb_pair[128,256] where wb holds bias/128 replicated down the rows, so
the row-sum reconstructs the bias. Keeping every matmul a uniform
K=128 fp16 op matters: mixing in K=1 bias matmuls was measured to hold
the PE at its low p-state (386ns per 256-col matmul instead of 109ns
back-to-back), and Pool/GpSimd cannot read PSUM on TRN2, so a
DVE-side bias add would bottleneck evacuation. PSUM evacuation is a
pure fp16 cast-copy split between ACT and DVE. x loads ride the SP
(sync) HWDGE ring; output stores alternate between the ACT and SP
rings.
"""

import os

# recover cleanly if a previous run left the NeuronCores wedged; must be
# set before the runtime initializes
os.environ.setdefault("NEURON_RT_RESET_CORES", "1")

import numpy as np

import concourse.bass as bass
import concourse.tile as tile
from concourse import bacc, mybir
from concourse.bass_utils import run_bass_kernel_spmd

N_CORES = 8
N, C, HI, HO = 4096, 256, 64, 128
CLOC = C // N_CORES   # 32 channels per core
PAIRS = CLOC // 2     # 16 channel pairs per core
NT = 128              # batch rows per tile
NTILES = N // NT      # 32 tiles

F32 = mybir.dt.float32
F16 = mybir.dt.float16


def build(n_cores=N_CORES):
    nc = bacc.Bacc(
        "TRN2", target_bir_lowering=False, debug=False, num_devices=n_cores
    )
    x_d = nc.dram_tensor(
        "xt", [128, NTILES, PAIRS, NT], F16, kind="ExternalInput"
    ).ap()
    w_d = nc.dram_tensor("wt", [128, PAIRS, 2 * HO], F16, kind="ExternalInput").ap()
    b_d = nc.dram_tensor("wb", [128, PAIRS, 2 * HO], F16, kind="ExternalInput").ap()
    o_d = nc.dram_tensor("out", [N, CLOC, HO], F16, kind="ExternalOutput").ap()

    with tile.TileContext(nc) as tc:
        with (
            tc.tile_pool(name="const", bufs=1) as const,
            tc.tile_pool(name="xp", bufs=4) as xp,
            tc.tile_pool(name="op", bufs=4) as op,
            tc.tile_pool(name="ps", bufs=8, space="PSUM") as psp,
        ):
            # ones first: no dependencies, so the GpSimd queue finishes it
            # immediately and the first bias matmul never waits on it
            ones = const.tile([128, NT], F16)
            nc.gpsimd.memset(ones, 1.0)
            # weights ride both HWDGE rings concurrently at startup, ahead
            # of the x loads (nothing can start until they land)
            wt = const.tile([128, PAIRS, 2 * HO], F16)
            nc.sync.dma_start(out=wt, in_=w_d)
            wb = const.tile([128, PAIRS, 2 * HO], F16)
            nc.scalar.dma_start(out=wb, in_=b_d)

            for t in range(NTILES):
                x_sb = xp.tile([128, PAIRS, NT], F16, name=f"x{t}", tag="x")
                nc.sync.dma_start(out=x_sb, in_=x_d[:, t, :, :])
                o_sb = op.tile([128, CLOC, HO], F16, name=f"o{t}", tag="o")
                for g in range(PAIRS // 2):  # 2 pairs (4 channels) per bank
                    po = psp.tile([128, 4, HO], F32)
                    for p in range(2):
                        j = 2 * g + p
                        out_ap = po[:, 2 * p : 2 * p + 2, :]
                        nc.tensor.matmul(
                            out_ap, lhsT=ones, rhs=wb[:, j, :],
                            start=True, stop=False,
                        )
                        nc.tensor.matmul(
                            out_ap, lhsT=x_sb[:, j, :], rhs=wt[:, j, :],
                            start=False, stop=True,
                        )
                    if g % 2 == 0:
                        nc.scalar.copy(out=o_sb[:, 4 * g : 4 * g + 4, :], in_=po)
                    else:
                        nc.vector.tensor_copy(
                            out=o_sb[:, 4 * g : 4 * g + 4, :], in_=po
                        )
                nc.scalar.dma_start(
                    out=o_d[t * NT : (t + 1) * NT, :, :], in_=o_sb
                )
    nc.compile()
    return nc


def pack_x(x):
    """[N, C, HI] f32 -> per-core [128, NTILES, PAIRS, NT] fp16.

    Partition k<64 holds channel (2p)'s tap k; k>=64 holds channel
    (2p+1)'s tap k-64, pre-transposed so lhsT slices DMA straight in.
    Returns one contiguous [N_CORES, 128, NTILES, PAIRS, NT] array.
    """
    v = x.reshape(NTILES, NT, N_CORES, PAIRS, 2, HI).astype(np.float16)
    # [t, n, core, p, e, i] -> [core, (e,i)=k, t, p, n]
    return np.ascontiguousarray(v.transpose(2, 4, 5, 0, 3, 1)).reshape(
        N_CORES, 128, NTILES, PAIRS, NT
    )


def pack_w(W):
    """[C, HO, HI] f32 -> per-core block-diag [128, PAIRS, 2*HO] fp16."""
    Wv = W.astype(np.float16).reshape(N_CORES, PAIRS, 2, HO, HI)
    out = np.zeros((N_CORES, 128, PAIRS, 2 * HO), dtype=np.float16)
    # upper-left: even channel of the pair, rows k=i, cols 0:HO
    out[:, :HI, :, :HO] = Wv[:, :, 0].transpose(0, 3, 1, 2)
    # lower-right: odd channel, rows k=64+i, cols HO:2HO
    out[:, HI:, :, HO:] = Wv[:, :, 1].transpose(0, 3, 1, 2)
    return out


def pack_b(b):
    """[C, HO] f32 -> per-core [128, PAIRS, 2*HO] fp16 "bias weights".

    Row k of wb holds bias/128 for the pair's concatenated [HO_even |
    HO_odd] outputs; ones.T @ wb sums the 128 rows back to the bias.
    """
    bp = (b.astype(np.float32) / 128.0).astype(np.float16)
    bp = bp.reshape(N_CORES, 1, PAIRS, 2 * HO)
    return np.ascontiguousarray(
        np.broadcast_to(bp, (N_CORES, 128, PAIRS, 2 * HO))
    )


_cache = {}


def kernel(x, W, b):
    nc = _cache.get("nc")
    if nc is None:
        nc = _cache["nc"] = build()
    xt = pack_x(np.asarray(x, dtype=np.float32))
    wt = pack_w(np.asarray(W, dtype=np.float32))
    bp = pack_b(np.asarray(b, dtype=np.float32))
    in_maps = [
        {"xt": xt[i], "wt": wt[i], "wb": bp[i]} for i in range(N_CORES)
    ]
    res = run_bass_kernel_spmd(nc, in_maps, core_ids=list(range(N_CORES)))
    out = np.empty((N, C, HO), dtype=np.float32)
    for i in range(N_CORES):
        out[:, i * CLOC : (i + 1) * CLOC, :] = res.results[i]["out"]
    return out


# revision 22
# speedup vs baseline: 1.1548x; 1.1548x over previous
"""Depthwise 1d (per-channel linear) Trainium2 Bass kernel.

out[n, c, o] = sum_i x[n, c, i] * W[c, o, i] + b[c, o]
  x: [4096, 256, 64] f32, W: [256, 128, 64] f32, b: [256, 128] f32
  out: [4096, 256, 128] f32

Strategy: shard channels C across 8 cores (32 ch/core, full batch).
Channels are fully independent, so there are no collectives; sharding C
instead of N means each core only needs 1/8th of the weights.

The kernel is DMA-bound (per-core HBM ~332 GB/s), so the whole design
minimizes HBM bytes:
  * x is downcast to fp16 AND pre-transposed on the host into the exact
    SBUF layout the PE wants: [k, tile, pair, n] where partitions k<64
    hold channel 2p's 64 taps and k>=64 hold channel 2p+1's. No on-device
    transposes, no hi/lo split -- halves x traffic vs f32.
  * the output is stored as fp16 and upcast to f32 on the host -- halves
    out traffic. (Error budget: gate is 2e-2; fp16 end-to-end is ~1.4e-3.)
  * weights are uploaded as the fully-assembled block-diagonal fp16 tiles
    [k, pair, 2*HO] (upper-left = W_even.T, lower-right = W_odd.T) so one
    matmul per channel pair contracts the full 128 partitions.

Per n-tile (128 rows) x pair: out_pair = xt_pair.T @ wt_pair, a
[128,128]x[128,256] fp16 matmul accumulated in fp32 PSUM. Each pair's
accumulation group is opened by a bias matmul: ones[128,128].T @
wb_pair[128,256] where wb holds bias/128 replicated down the rows, so
the row-sum reconstructs the bias. Keeping every matmul a uniform
K=128 fp16 op matters: mixing in K=1 bias matmuls was measured to hold
the PE at its low p-state (386ns per 256-col matmul instead of 109ns
back-to-back), and Pool/GpSimd cannot read PSUM on TRN2, so a
DVE-side bias add would bottleneck evacuation. PSUM evacuation is a
pure fp16 cast-copy split between ACT and DVE. x loads ride the SP
(sync) HWDGE ring; output stores alternate between the ACT and SP
rings.
"""

import os

# recover cleanly if a previous run left the NeuronCores wedged; must be
# set before the runtime initializes
os.environ.setdefault("NEURON_RT_RESET_CORES", "1")

import numpy as np

import concourse.bass as bass
import concourse.tile as tile
from concourse import bacc, mybir
from concourse.bass_utils import run_bass_kernel_spmd

N_CORES = 8
N, C, HI, HO = 4096, 256, 64, 128
CLOC = C // N_CORES   # 32 channels per core
PAIRS = CLOC // 2     # 16 channel pairs per core
NT = 128              # batch rows per tile
NTILES = N // NT      # 32 tiles

F32 = mybir.dt.float32
F16 = mybir.dt.float16


def build(n_cores=N_CORES):
    nc = bacc.Bacc(
        "TRN2", target_bir_lowering=False, debug=False, num_devices=n_cores
    )
    x_d = nc.dram_tensor(
        "xt", [128, NTILES, PAIRS, NT], F16, kind="ExternalInput"
    ).ap()
    w_d = nc.dram_tensor("wt", [128, PAIRS, 2 * HO], F16, kind="ExternalInput").ap()
    b_d = nc.dram_tensor("wb", [128, PAIRS, 2 * HO], F16, kind="ExternalInput").ap()
    o_d = nc.dram_tensor("out", [N, CLOC, HO], F16, kind="ExternalOutput").ap()

    with tile.TileContext(nc) as tc:
        with (
            tc.tile_pool(name="const", bufs=1) as const,
            tc.tile_pool(name="xp", bufs=4) as xp,
            tc.tile_pool(name="op", bufs=4) as op,
            tc.tile_pool(name="ps", bufs=8, space="PSUM") as psp,
        ):
            # ones first: no dependencies, so the GpSimd queue finishes it
            # immediately and the first bias matmul never waits on it
            ones = const.tile([128, NT], F16)
            nc.gpsimd.memset(ones, 1.0)
            # weights ride both HWDGE rings concurrently at startup, ahead
            # of the x loads (nothing can start until they land)
            wt = const.tile([128, PAIRS, 2 * HO], F16)
            nc.sync.dma_start(out=wt, in_=w_d)
            wb = const.tile([128, PAIRS, 2 * HO], F16)
            nc.scalar.dma_start(out=wb, in_=b_d)

            for t in range(NTILES):
                x_sb = xp.tile([128, PAIRS, NT], F16, name=f"x{t}", tag="x")
                nc.sync.dma_start(out=x_sb, in_=x_d[:, t, :, :])
                o_sb = op.tile([128, CLOC, HO], F16, name=f"o{t}", tag="o")
                for g in range(PAIRS // 2):  # 2 pairs (4 channels) per bank
                    po = psp.tile([128, 4, HO], F32)
                    for p in range(2):
                        j = 2 * g + p
                        out_ap = po[:, 2 * p : 2 * p + 2, :]
                        nc.tensor.matmul(
                            out_ap, lhsT=ones, rhs=wb[:, j, :],
                            start=True, stop=False,
                        )
                        nc.tensor.matmul(
                            out_ap, lhsT=x_sb[:, j, :], rhs=wt[:, j, :],
                            start=False, stop=True,
                        )
                    if g % 2 == 0:
                        nc.scalar.copy(out=o_sb[:, 4 * g : 4 * g + 4, :], in_=po)
                    else:
                        nc.vector.tensor_copy(
                            out=o_sb[:, 4 * g : 4 * g + 4, :], in_=po
                        )
                # stores ride the SWDGE ring: the idle Pool engine issues
                # them, so the wait on the 8 copies never head-of-line
                # blocks the ACT/SP queues that feed the next iteration
                nc.gpsimd.dma_start(
                    out=o_d[t * NT : (t + 1) * NT, :, :], in_=o_sb
                )
    nc.compile()
    return nc


def pack_x(x):
    """[N, C, HI] f32 -> per-core [128, NTILES, PAIRS, NT] fp16.

    Partition k<64 holds channel (2p)'s tap k; k>=64 holds channel
    (2p+1)'s tap k-64, pre-transposed so lhsT slices DMA straight in.
    Returns one contiguous [N_CORES, 128, NTILES, PAIRS, NT] array.
    """
    v = x.reshape(NTILES, NT, N_CORES, PAIRS, 2, HI).astype(np.float16)
    # [t, n, core, p, e, i] -> [core, (e,i)=k, t, p, n]
    return np.ascontiguousarray(v.transpose(2, 4, 5, 0, 3, 1)).reshape(
        N_CORES, 128, NTILES, PAIRS, NT
    )


def pack_w(W):
    """[C, HO, HI] f32 -> per-core block-diag [128, PAIRS, 2*HO] fp16."""
    Wv = W.astype(np.float16).reshape(N_CORES, PAIRS, 2, HO, HI)
    out = np.zeros((N_CORES, 128, PAIRS, 2 * HO), dtype=np.float16)
    # upper-left: even channel of the pair, rows k=i, cols 0:HO
    out[:, :HI, :, :HO] = Wv[:, :, 0].transpose(0, 3, 1, 2)
    # lower-right: odd channel, rows k=64+i, cols HO:2HO
    out[:, HI:, :, HO:] = Wv[:, :, 1].transpose(0, 3, 1, 2)
    return out


def pack_b(b):
    """[C, HO] f32 -> per-core [128, PAIRS, 2*HO] fp16 "bias weights".

    Row k of wb holds bias/128 for the pair's concatenated [HO_even |
    HO_odd] outputs; ones.T @ wb sums the 128 rows back to the bias.
    """
    bp = (b.astype(np.float32) / 128.0).astype(np.float16)
    bp = bp.reshape(N_CORES, 1, PAIRS, 2 * HO)
    return np.ascontiguousarray(
        np.broadcast_to(bp, (N_CORES, 128, PAIRS, 2 * HO))
    )


_cache = {}


def kernel(x, W, b):
    nc = _cache.get("nc")
    if nc is None:
        nc = _cache["nc"] = build()
    xt = pack_x(np.asarray(x, dtype=np.float32))
    wt = pack_w(np.asarray(W, dtype=np.float32))
    bp = pack_b(np.asarray(b, dtype=np.float32))
    in_maps = [
        {"xt": xt[i], "wt": wt[i], "wb": bp[i]} for i in range(N_CORES)
    ]
    res = run_bass_kernel_spmd(nc, in_maps, core_ids=list(range(N_CORES)))
    out = np.empty((N, C, HO), dtype=np.float32)
    for i in range(N_CORES):
        out[:, i * CLOC : (i + 1) * CLOC, :] = res.results[i]["out"]
    return out
